# revision 19
# baseline (speedup 1.0000x reference)
"""Trainium2 Bass kernel for CronRootAttention (sparse attention).

Shapes (hardcoded): B=2 H=16 S=4096 D=128, W=64, NB=R=64.
Sharding: fused B*H=32 axis split across 8 cores (4 slices/core).

v3 design: scores are computed TRANSPOSED (sT[key, query]) so the exp
output is already in the layout PV needs as its stationary operand.
Per (b,h) slice:
  - local QK: one matmul per 128-key block: sT[128k, 192q]
    (key block stationary, queries moving); the causal window mask is a
    single constant [128,192] 0/1 multiply, identical for every block
  - strided+relay QK: batched into 8 matmuls [128sr, 512q] per slice
    (kTsr stationary is tile-independent); boundary masks are tiny
    [3,128] 0/1 multiplies per query tile
  - ACT exp (scale folded) -> pT bf16 straight into SBUF; no PE
    transposes, no mask matmuls, no PSUM->SBUF copy
  - PV per 128-query tile: 3 accumulating matmuls (block i, tail of
    block i-1 into partitions 0:64, sr keys) vs v tiles augmented with
    a ones-column so out[:,128] is the joint softmax denominator
  - DVE reciprocal + per-partition scale, DMA out (fp32)
  - emission is software-pipelined depth-2 so the PE never waits on
    ACT/GPSIMD inside a tile
"""

import numpy as np
import ml_dtypes

import concourse.bass as bass
import concourse.bacc as bacc
import concourse.tile as tile
from concourse import mybir
from concourse.bass_utils import run_bass_kernel_spmd

BF16 = ml_dtypes.bfloat16
B, H, S, D = 2, 16, 4096, 128
W = 64
NB = S // W          # 64
R = NB               # 64
NCORES = 8
SLICES = B * H // NCORES   # 4
NT = S // 128        # 32 query tiles (= key blocks) per slice
SCALE = 1.0 / np.sqrt(np.float32(D))
DV = D + 1           # v columns + ones column

_prog_cache = {}


def _build_masks():
    # band01[t, jj]: key t (within block b) valid for query jj (offset from
    # block start 128b) iff jj-63 <= t <= jj.  Same for every block.
    t = np.arange(128)[:, None]
    jj = np.arange(192)[None, :]
    band01 = ((jj - 63 <= t) & (t <= jj)).astype(np.float32)
    # global sr validity mask over (interleaved sr row p, absolute query m):
    #   str row p=2s valid iff m >= 64s+64 = 32p+64
    #   rel row p=2s+1 valid iff m >= 64s+127 = 32(p-1)+127 = 32p+95
    p = np.arange(128)[:, None]
    m = np.arange(S)[None, :]
    thr = 32 * p + np.where(p % 2 == 0, 64, 95)
    srm01 = (m >= thr).astype(np.float32)
    return band01.astype(BF16), srm01.astype(BF16)


def build_program():
    if "nc" in _prog_cache:
        return _prog_cache["nc"]
    dt = mybir.dt
    nc = bacc.Bacc("TRN2", target_bir_lowering=False, debug=False)

    qT_d = nc.declare_dram_parameter("qT", [SLICES, 128, S], dt.bfloat16, isOutput=False)
    kT_d = nc.declare_dram_parameter("kT", [SLICES, 128, S], dt.bfloat16, isOutput=False)
    vb_d = nc.declare_dram_parameter("vb", [SLICES, 128, NT * DV], dt.bfloat16, isOutput=False)
    kTsr_d = nc.declare_dram_parameter("kTsr", [SLICES, 128, 128], dt.bfloat16, isOutput=False)
    vsr_d = nc.declare_dram_parameter("vsr", [SLICES, 128, DV], dt.bfloat16, isOutput=False)
    band_d = nc.declare_dram_parameter("band", [128, 192], dt.bfloat16, isOutput=False)
    srm_d = nc.declare_dram_parameter("srm", [128, S], dt.bfloat16, isOutput=False)
    # out[s, c, p, 128*t + d] = O[s, 512*c + 128*t + p, d]; un-permuted on host
    out_d = nc.declare_dram_parameter("out", [SLICES, 8, 128, 512], dt.float32, isOutput=True)

    from contextlib import ExitStack
    with tile.TileContext(nc) as tc, ExitStack() as ctx:
        cpool = ctx.enter_context(tc.tile_pool(name="consts", bufs=1))
        band = cpool.tile([128, 192], dt.bfloat16, tag="band")
        nc.sync.dma_start(band[:], band_d[:, :])
        srm = cpool.tile([128, S], dt.bfloat16, tag="srm")
        nc.sync.dma_start(srm[:], srm_d[:, :])

        spool = ctx.enter_context(tc.tile_pool(name="slice_in", bufs=2))
        srp = ctx.enter_context(tc.tile_pool(name="psrT", bufs=2))
        psr = ctx.enter_context(tc.tile_pool(name="psum_sr", bufs=2, space="PSUM"))
        ploc = ctx.enter_context(tc.tile_pool(name="psum_loc", bufs=3, space="PSUM"))
        pout = ctx.enter_context(tc.tile_pool(name="psum_out", bufs=3, space="PSUM"))
        plsb = ctx.enter_context(tc.tile_pool(name="p_sb", bufs=5))
        wout = ctx.enter_context(tc.tile_pool(name="wout", bufs=2))

        state = {}
        ostages = {}

        def front(sc, s, b):
            if b % 4 == 0:
                c = b // 4
                sch = psr.tile([128, 512], dt.float32, tag="sch")
                nc.tensor.matmul(sch[:], sc["kTsr"][:],
                                 sc["qT"][:, 512 * c:512 * (c + 1)],
                                 start=True, stop=True)
                nc.scalar.activation(sc["p_srT"][:, 512 * c:512 * (c + 1)], sch[:],
                                     mybir.ActivationFunctionType.Exp, scale=float(SCALE))
                nc.vector.tensor_mul(sc["p_srT"][:, 512 * c:512 * (c + 1)],
                                     sc["p_srT"][:, 512 * c:512 * (c + 1)],
                                     srm[:, 512 * c:512 * (c + 1)])
            qw = 192 if b < NT - 1 else 128
            sb_ = ploc.tile([128, 192], dt.float32, tag="sT")
            nc.tensor.matmul(sb_[:, 0:qw], sc["kT"][:, 128 * b:128 * b + 128],
                             sc["qT"][:, 128 * b:128 * b + qw], start=True, stop=True)
            p_b = plsb.tile([128, 192], dt.bfloat16, tag="p_b")
            nc.scalar.activation(p_b[:, 0:qw], sb_[:, 0:qw],
                                 mybir.ActivationFunctionType.Exp, scale=float(SCALE))
            nc.gpsimd.tensor_mul(p_b[:, 0:qw], p_b[:, 0:qw], band[:, 0:qw])
            prev = state[(s, b - 1)][0] if b > 0 else None
            state[(s, b)] = (p_b, prev, sc["vb"], sc["vsr"], sc["p_srT"])

        def back(s, b):
            p_b, p_prev, vb, vsr, p_srT = state[(s, b)]
            outp = pout.tile([128, DV], dt.float32, tag="outp")
            nc.tensor.matmul(outp[:], p_b[:, 0:128], vb[:, DV * b:DV * (b + 1)],
                             start=True, stop=False)
            if b > 0:
                nc.tensor.matmul(outp[0:64, :], p_prev[:, 128:192],
                                 vb[:, DV * (b - 1):DV * b],
                                 start=False, stop=False, skip_group_check=True)
            n2 = 4 * b + 2
            nc.tensor.matmul(outp[:], p_srT[0:n2, 128 * b:128 * b + 128], vsr[0:n2, :],
                             start=False, stop=True, skip_group_check=True)
            rsum = wout.tile([128, 1], dt.float32, tag="rsum")
            nc.vector.reciprocal(rsum[:], outp[:, 128:129])
            t = b % 4
            if t == 0:
                ostage = wout.tile([128, 512], dt.float32, tag="ostage")
                ostages[(s, b // 4)] = ostage
            ostage = ostages[(s, b // 4)]
            nc.vector.tensor_scalar_mul(ostage[:, 128 * t:128 * (t + 1)],
                                        outp[:, 0:128], rsum[:])
            if t == 3:
                nc.sync.dma_start(out_d[s, b // 4], ostage[:])
                ostages.pop((s, b // 4))
            if b >= 2:
                state.pop((s, b - 2))

        def alloc_slice():
            qT = spool.tile([128, S], dt.bfloat16, tag="qT")
            kT = spool.tile([128, S], dt.bfloat16, tag="kT")
            vb = spool.tile([128, NT * DV], dt.bfloat16, tag="vb")
            kTsr = spool.tile([128, 128], dt.bfloat16, tag="kTsr")
            vsr = spool.tile([128, DV], dt.bfloat16, tag="vsr")
            p_srT = srp.tile([128, S], dt.bfloat16, tag="p_srT")
            return dict(qT=qT, kT=kT, vb=vb, kTsr=kTsr, vsr=vsr, p_srT=p_srT)

        def slice_dmas(s, t):
            d = [(t["kTsr"][:], kTsr_d[s]),
                 (t["kT"][:, 0:256], kT_d[s, :, 0:256]),
                 (t["qT"][:, 0:512], qT_d[s, :, 0:512]),
                 (t["vsr"][:], vsr_d[s]),
                 (t["vb"][:, 0:4 * DV], vb_d[s, :, 0:4 * DV]),
                 (t["qT"][:, 512:1024], qT_d[s, :, 512:1024]),
                 (t["kT"][:, 256:1024], kT_d[s, :, 256:1024]),
                 (t["vb"][:, 4 * DV:16 * DV], vb_d[s, :, 4 * DV:16 * DV]),
                 (t["qT"][:, 1024:2048], qT_d[s, :, 1024:2048]),
                 (t["kT"][:, 1024:2048], kT_d[s, :, 1024:2048]),
                 (t["vb"][:, 16 * DV:NT * DV], vb_d[s, :, 16 * DV:NT * DV]),
                 (t["qT"][:, 2048:4096], qT_d[s, :, 2048:4096]),
                 (t["kT"][:, 2048:4096], kT_d[s, :, 2048:4096])]
            return d

        nxt = alloc_slice()
        for dst, src in slice_dmas(0, nxt):
            nc.sync.dma_start(dst, src)
        slice_ctx = [None] * SLICES
        pend = []
        TOT = NT * SLICES
        for t in range(TOT + 2):
            if t < TOT:
                s, b = divmod(t, NT)
                if b == 0:
                    slice_ctx[s] = nxt
                    pend = slice_dmas(s + 1, nxt := alloc_slice()) \
                        if s + 1 < SLICES else []
                front(slice_ctx[s], s, b)
                if 4 <= b < 4 + len(pend):
                    dst, src = pend[b - 4]
                    nc.sync.dma_start(dst, src)
            if t >= 2:
                tb = t - 2
                back(tb // NT, tb % NT)

    nc.finalize()
    _prog_cache["nc"] = nc
    return nc


def _prep_core_inputs(q, k, v, rk, rv, masks):
    """q,k,v: [SLICES, S, D] fp32 for one core; rk, rv: [SLICES, R, D]."""
    band01, srm01 = masks
    qb = q.astype(BF16)
    kb = k.astype(BF16)
    vf = v.astype(BF16)
    qT = np.ascontiguousarray(qb.transpose(0, 2, 1))          # [SL, 128, S]
    kT = np.ascontiguousarray(kb.transpose(0, 2, 1))
    # blocked v with ones column, key-partition layout: vb[s, t, b*DV+d]
    vblk = vf.reshape(SLICES, NT, 128, D).transpose(0, 2, 1, 3)   # [SL,128,NT,D]
    vblk = np.concatenate([vblk, np.ones((SLICES, 128, NT, 1), BF16)], axis=3)
    vb = np.ascontiguousarray(vblk.reshape(SLICES, 128, NT * DV))
    # interleaved strided/relay keys: row 2j = k[64j], row 2j+1 = rk[j]
    ksr = np.empty((SLICES, 128, D), BF16)
    ksr[:, 0::2] = kb[:, ::W, :]
    ksr[:, 1::2] = rk.astype(BF16)
    kTsr = np.ascontiguousarray(ksr.transpose(0, 2, 1))           # [SL, 128, 128]
    vsr_pairs = np.empty((SLICES, 128, D), BF16)
    vsr_pairs[:, 0::2] = vf[:, ::W, :]
    vsr_pairs[:, 1::2] = rv.astype(BF16)
    vsr = np.ascontiguousarray(
        np.concatenate([vsr_pairs, np.ones((SLICES, 128, 1), BF16)], axis=2))
    return {
        "qT": qT, "kT": kT, "vb": vb, "kTsr": kTsr, "vsr": vsr,
        "band": band01, "srm": srm01,
    }


def make_in_maps(q, k, v, rk, rv):
    masks = _build_masks()
    qf = q.reshape(B * H, S, D)
    kf = k.reshape(B * H, S, D)
    vf = v.reshape(B * H, S, D)
    rkf = rk.reshape(B * H, R, D)
    rvf = rv.reshape(B * H, R, D)
    in_maps = []
    for c in range(NCORES):
        sl = slice(SLICES * c, SLICES * (c + 1))
        in_maps.append(_prep_core_inputs(qf[sl], kf[sl], vf[sl], rkf[sl], rvf[sl],
                                         masks))
    return in_maps


def kernel(q, k, v, rk, rv, _run_kwargs=None):
    q = np.asarray(q, dtype=np.float32)
    k = np.asarray(k, dtype=np.float32)
    v = np.asarray(v, dtype=np.float32)
    rk = np.asarray(rk, dtype=np.float32)
    rv = np.asarray(rv, dtype=np.float32)
    nc = build_program()
    in_maps = make_in_maps(q, k, v, rk, rv)
    res = run_bass_kernel_spmd(nc, in_maps, list(range(NCORES)), **(_run_kwargs or {}))
    out = np.stack([res.results[c]["out"] for c in range(NCORES)])  # [8, SL, 8, 128, 512]
    if _run_kwargs:
        kernel.last_results = res
    # out[core, s, c, p, 128*t + d] = O[core, s, 512*c + 128*t + p, d]
    out = out.reshape(NCORES, SLICES, 8, 128, 4, D).transpose(0, 1, 2, 4, 3, 5)
    return np.ascontiguousarray(out).reshape(B, H, S, D)
